# revision 2
# baseline (speedup 1.0000x reference)
"""CfC cell (dense MLP) Trainium2 Bass kernel.

Reference math (fp32):
    x  = concat([input, hx], axis=1)                  # [B, 768]
    h  = 1.7159 * tanh(0.666 * (x @ Wb.T + bb))       # [B, 1024]
    ff1 = tanh(h @ W1.T + b1)                         # [B, 512]
    ff2 = tanh(h @ W2.T + b2)
    t_a = h @ Wa.T + ba
    t_b = h @ Wt.T + bt
    t   = sigmoid(t_a * ts + t_b)
    out = ff1 * (1 - t) + t * ff2

Strategy: data-parallel over batch across 8 NeuronCores (2048 rows each).
Mixed precision: the backbone and the two tanh heads (ff1/ff2) run in fp16
(their error feeds the output through tanh with gain ~1), while the two
t-path heads (Wa/Wt) run in fp8-e4m3 with DoubleRow matmuls (2x PE rate,
half the instructions) — their quantization noise is squashed by
sigmoid' <= 0.25 and measured end-to-end rel-fro error stays ~1.5e-2.

Layouts (host-side prep):
  - xT        [768, 2048]    (x transposed; contraction dim on partitions)
  - WbT       [768, 1024]    (Wb.T fp16; stationary lhsT tiles for layer 1)
  - WH        [2, 1024, 512] (1.7159 * Wk.T fp16, k in {1,2}; moving rhs)
  - WT8       [2, 128, 8, 512] fp8 (1024*1.7159*Wk.T, k in {a,t}; slab-major
                              DoubleRow layout: k_global = slab*128 + p)
  - BBP       [128, 8]       (0.666*bb, per unit-tile columns; ACT bias)
  - BH        [4, 128, 512]  (head biases broadcast across partitions)
  - TSP       [128, 16]      (ts, column mi = batch subtile mi)
Layer 1 produces g = tanh(0.666*z) tiles [units, batch] in fp16 (ff
stationary) and a packed fp8 copy [128, 8, 512] (t-head stationary).
Layer 2 emits [batch, hid] tiles so ts is a per-partition scalar and the
result DMAs out with no transpose.
"""

import os
import sys

import numpy as np
import ml_dtypes

if "/opt/trn_rl_repo" not in sys.path:
    sys.path.insert(0, "/opt/trn_rl_repo")

B, IN, HID, UNITS = 16384, 256, 512, 1024
CAT = IN + HID  # 768
N_CORES = 8
BS = B // N_CORES  # 2048 per core
P = 128
NK1 = CAT // P    # 6 contraction tiles, layer 1
NU = UNITS // P   # 8 unit tiles
SW8 = 1024.0      # fp8 scale on the t-path head weights

_cache = {}


def build_nc(bs=BS, chunk=512):
    """Build the single-core Bass program (same program runs SPMD on 8 cores)."""
    from concourse import bacc, tile, mybir

    AF = mybir.ActivationFunctionType
    ALU = mybir.AluOpType
    F32 = mybir.dt.float32
    F16 = mybir.dt.float16
    F8 = mybir.dt.float8e4
    DR = mybir.MatmulPerfMode.DoubleRow

    nchunk = bs // chunk
    nm = chunk // P  # batch subtiles per chunk

    nc = bacc.Bacc("TRN2", target_bir_lowering=False, debug=False,
                   num_devices=N_CORES)

    xt_d = nc.dram_tensor("xt", [CAT, bs], F16, kind="ExternalInput").ap()
    wbt_d = nc.dram_tensor("wbt", [CAT, UNITS], F16, kind="ExternalInput").ap()
    wh_d = nc.dram_tensor("wh", [2, UNITS, HID], F16, kind="ExternalInput").ap()
    wt8_d = nc.dram_tensor("wt8", [2, P, NU, HID], F8, kind="ExternalInput").ap()
    bbp_d = nc.dram_tensor("bbp", [P, NU], F32, kind="ExternalInput").ap()
    bh_d = nc.dram_tensor("bh", [4, P, HID], F32, kind="ExternalInput").ap()
    tsp_d = nc.dram_tensor("tsp", [P, bs // P], F32, kind="ExternalInput").ap()
    out_d = nc.dram_tensor("out", [bs, HID], F32, kind="ExternalOutput").ap()

    with tile.TileContext(nc) as tc:
        with (
            tc.tile_pool(name="const", bufs=1) as const,
            tc.tile_pool(name="xp", bufs=4) as xp,
            tc.tile_pool(name="hp", bufs=4) as hp,
            tc.tile_pool(name="tp", bufs=2) as tp,
            tc.tile_pool(name="op", bufs=3) as op,
            tc.tile_pool(name="psp", bufs=8, space="PSUM") as psp,
        ):
            # --- PE warmup: keep HAM busy while startup DMAs stream ------
            warm = const.tile([P, 512], F16, tag="warm")
            nc.gpsimd.memset(warm[:], 0.0)
            for _ in range(8):
                wps = psp.tile([P, 512], F32, tag="ps")
                nc.tensor.matmul(wps[:], warm[:, 0:P], warm[:],
                                 start=True, stop=True)

            def load_x(bc):
                xts = []
                for c in range(NK1):
                    t = xp.tile([P, chunk], F16, tag=f"x{c}")
                    nc.sync.dma_start(
                        t[:], xt_d[c * P:(c + 1) * P, bc * chunk:(bc + 1) * chunk])
                    xts.append(t)
                return xts

            # first-chunk x tiles interleaved with the first weight half so
            # the c=0 accumulation group is runnable almost immediately
            HALF = UNITS // 2
            wb_sb = [[None, None] for _ in range(NK1)]
            xts0 = []
            for c in range(NK1):
                t = xp.tile([P, chunk], F16, tag=f"x{c}")
                nc.sync.dma_start(t[:], xt_d[c * P:(c + 1) * P, 0:chunk])
                xts0.append(t)
                t = const.tile([P, HALF], F16, tag=f"wbh{c}_0")
                nc.sync.dma_start(t[:], wbt_d[c * P:(c + 1) * P, 0:HALF])
                wb_sb[c][0] = t

            # small constants early (bb gates every layer-1 activation)
            bb_sb = const.tile([P, NU], F32, tag="bb")
            nc.sync.dma_start(bb_sb[:], bbp_d[:])
            ts_sb = const.tile([P, bs // P], F32, tag="ts")
            nc.sync.dma_start(ts_sb[:], tsp_d[:])

            for c in range(NK1):
                t = const.tile([P, HALF], F16, tag=f"wbh{c}_1")
                nc.sync.dma_start(
                    t[:], wbt_d[c * P:(c + 1) * P, HALF:UNITS])
                wb_sb[c][1] = t

            # all remaining x chunks next: layer-1 for every chunk runs
            # before any layer-2, so the head weights are needed only ~50us in
            xts_all = [xts0] + [load_x(bc) for bc in range(1, nchunk)]

            bh_sb = [None] * 4
            for k in range(4):
                t = const.tile([P, HID], F32, tag=f"bh{k}", name=f"bh_{k}")
                nc.sync.dma_start(t[:], bh_d[k])
                bh_sb[k] = t

            # t-path fp8 head weights: slab-major DoubleRow layout, one DMA each
            wt8_sb = [None, None]
            for k in range(2):
                t = const.tile([P, NU, HID], F8, tag=f"wt8_{k}", name=f"wt8_{k}")
                nc.sync.dma_start(t[:], wt8_d[k])
                wt8_sb[k] = t

            wh_sb = [None, None]

            def load_wh(k):
                row = []
                for u in range(NU):
                    t = const.tile([P, HID], F16, tag=f"wh{k}_{u}",
                                   name=f"wh_{k}_{u}")
                    nc.sync.dma_start(t[:], wh_d[k, u * P:(u + 1) * P, :])
                    row.append(t)
                wh_sb[k] = row

            load_wh(0)
            load_wh(1)

            # per-chunk fp8 copy of g (t-head stationary), slab-major
            g8_all = [const.tile([P, NU, chunk], F8, tag=f"g8_{bc}",
                                 name=f"g8_{bc}")
                      for bc in range(nchunk)]

            def layer1(xts, bc):
                """g[u] = tanh(0.666*(WbT.T @ xT) + 0.666*bb), fp16 + fp8 out.

                c-outer accumulation in two u-half-groups: the first matmul
                only needs xts[0] + wb half, so PE starts as soon as the
                first ~0.26 MB of DMA lands.
                """
                g8 = g8_all[bc]
                hts = []
                for h in range(2):
                    pss = [psp.tile([P, chunk], F32, tag="ps", name=f"psl1_{j}")
                           for j in range(NU // 2)]
                    for c in range(NK1):
                        for j in range(NU // 2):
                            nc.tensor.matmul(
                                pss[j][:],
                                wb_sb[c][h][:, j * P:(j + 1) * P],
                                xts[c][:],
                                start=(c == 0), stop=(c == NK1 - 1))
                    for j in range(NU // 2):
                        u = h * (NU // 2) + j
                        ht = hp.tile([P, chunk], F16, tag=f"h{u}")
                        nc.scalar.activation(ht[:], pss[j][:], AF.Tanh,
                                             bias=bb_sb[:, u:u + 1], scale=0.666)
                        nc.vector.tensor_scalar_mul(g8[:, u, :], ht[:], 1.0)
                        hts.append(ht)
                return hts

            def layer2(hts, bc):
                g8 = g8_all[bc]
                for m in range(nm):
                    mi = bc * nm + m
                    last = (bc == nchunk - 1) and (m == nm - 1)
                    ms = slice(m * P, (m + 1) * P)

                    # t-path heads first (fp8 DoubleRow, shared stationary
                    # per u-pair) so the sigmoid chain overlaps the ff matmuls
                    pa = psp.tile([P, HID], F32, tag="ps", name="ps_a")
                    pb = psp.tile([P, HID], F32, tag="ps", name="ps_b")
                    for j in range(NU // 2):
                        sl = slice(2 * j, 2 * j + 2)
                        nc.tensor.matmul(pa[:], g8[:, sl, ms], wt8_sb[0][:, sl, :],
                                         start=(j == 0), stop=(j == NU // 2 - 1),
                                         perf_mode=DR)
                        nc.tensor.matmul(pb[:], g8[:, sl, ms], wt8_sb[1][:, sl, :],
                                         start=(j == 0), stop=(j == NU // 2 - 1),
                                         perf_mode=DR)
                    ua = tp.tile([P, HID], F32, tag="ua")
                    nc.vector.scalar_tensor_tensor(
                        ua[:], pa[:], 1.0 / SW8, bh_sb[2][:],
                        op0=ALU.mult, op1=ALU.add)
                    ub = tp.tile([P, HID], F32, tag="ub")
                    nc.vector.scalar_tensor_tensor(
                        ub[:], pb[:], 1.0 / SW8, bh_sb[3][:],
                        op0=ALU.mult, op1=ALU.add)
                    w = tp.tile([P, HID], F32, tag="w")
                    nc.vector.scalar_tensor_tensor(
                        w[:], ua[:], ts_sb[:, mi:mi + 1], ub[:],
                        op0=ALU.mult, op1=ALU.add)
                    tt = tp.tile([P, HID], F32, tag="tt")
                    nc.scalar.activation(tt[:], w[:], AF.Sigmoid)

                    # ff heads: shared stationary per u (ff1 then ff2)
                    p1 = psp.tile([P, HID], F32, tag="ps", name="ps_1")
                    p2 = psp.tile([P, HID], F32, tag="ps", name="ps_2")
                    for u in range(NU):
                        nc.tensor.matmul(p1[:], hts[u][:, ms], wh_sb[0][u][:],
                                         start=(u == 0), stop=(u == NU - 1))
                        nc.tensor.matmul(p2[:], hts[u][:, ms], wh_sb[1][u][:],
                                         start=(u == 0), stop=(u == NU - 1))
                    u1 = tp.tile([P, HID], F32, tag="u1")
                    nc.vector.tensor_add(u1[:], p1[:], bh_sb[0][:])
                    f1 = tp.tile([P, HID], F32, tag="f1")
                    nc.scalar.activation(f1[:], u1[:], AF.Tanh)

                    o = op.tile([P, HID], F32, tag="o")
                    f2 = tp.tile([P, HID], F32, tag="f2")
                    # split the trailing chain into column halves on the very
                    # last tile so ACT/DVE pipeline instead of serializing
                    cols = ((slice(0, HID // 2), slice(HID // 2, HID))
                            if last else (slice(0, HID),))
                    for cs in cols:
                        u2 = tp.tile([P, HID], F32, tag="u2")
                        nc.vector.tensor_add(u2[:, cs], p2[:, cs], bh_sb[1][:, cs])
                        nc.scalar.activation(f2[:, cs], u2[:, cs], AF.Tanh)
                        # o = f1 + tt*(f2 - f1)
                        nc.vector.tensor_sub(o[:, cs], f2[:, cs], f1[:, cs])
                        nc.vector.tensor_mul(o[:, cs], o[:, cs], tt[:, cs])
                        nc.vector.tensor_add(o[:, cs], o[:, cs], f1[:, cs])
                        nc.sync.dma_start(out_d[mi * P:(mi + 1) * P, cs], o[:, cs])

            # --- all layer-1 chunks first, then all layer-2 --------------
            hts_all = [layer1(x, bc) for bc, x in enumerate(xts_all)]
            for bc in range(nchunk):
                layer2(hts_all[bc], bc)

    nc.compile()
    return nc


def _prep_inputs(input, hx, ts, Wb, bb, W1, b1, W2, b2, Wa, ba, Wt, bt, bs=BS,
                 n_cores=N_CORES):
    f = np.float32
    h = np.float16
    e4 = ml_dtypes.float8_e4m3
    x = np.concatenate([np.asarray(input, f), np.asarray(hx, f)], axis=1)
    WbT = np.ascontiguousarray(np.asarray(Wb, f).T.astype(h))   # [768, 1024]
    WH = np.stack([np.ascontiguousarray((1.7159 * np.asarray(W, f)).T.astype(h))
                   for W in (W1, W2)])                          # [2, 1024, 512]
    # t-path heads: fp8, slab-major DoubleRow layout [128, 8, 512]
    WT8 = np.stack([
        np.ascontiguousarray(
            (SW8 * 1.7159 * np.asarray(W, f)).T
            .reshape(NU, P, HID).transpose(1, 0, 2).astype(e4))
        for W in (Wa, Wt)])                                     # [2, 128, 8, 512]
    BBP = np.ascontiguousarray(
        (0.666 * np.asarray(bb, f)).reshape(NU, P).T)           # [128, 8]
    BH = np.stack([np.ascontiguousarray(np.broadcast_to(np.asarray(b, f), (P, HID)))
                   for b in (b1, b2, ba, bt)])                  # [4, 128, 512]
    ts = np.asarray(ts, f).reshape(-1)
    xh = x.astype(h)

    in_maps = []
    for c in range(n_cores):
        lo, hi = c * bs, (c + 1) * bs
        in_maps.append({
            "xt": np.ascontiguousarray(xh[lo:hi].T),            # [768, bs] fp16
            "wbt": WbT,
            "wh": WH,
            "wt8": WT8,
            "bbp": BBP,
            "bh": BH,
            "tsp": np.ascontiguousarray(ts[lo:hi].reshape(bs // P, P).T),
        })
    return in_maps


def kernel(input, hx, ts, Wb, bb, W1, b1, W2, b2, Wa, ba, Wt, bt):
    from concourse.bass_utils import run_bass_kernel_spmd

    if "nc" not in _cache:
        _cache["nc"] = build_nc()
    nc = _cache["nc"]

    in_maps = _prep_inputs(input, hx, ts, Wb, bb, W1, b1, W2, b2, Wa, ba, Wt, bt)
    trace = bool(int(os.environ.get("KERNEL_PROFILE", "0")))
    res = run_bass_kernel_spmd(nc, in_maps, list(range(N_CORES)), trace=trace)
    _cache["last_exec_time_ns"] = res.exec_time_ns
    _cache["last_results"] = res

    out = np.concatenate([res.results[c]["out"] for c in range(N_CORES)], axis=0)
    return out.astype(np.float32)
